# revision 12
# baseline (speedup 1.0000x reference)
"""Trainium2 Bass kernel for nn_Attn: batched column-softmax attention energies.

Math (per batch element b):
    E = encoder_outputs[:, b, :]            # [H, T]
    d = decoder_hidden[b]                   # [H]
    s = E^T d                               # [T]  (scores)
    w[h, t] = E[h, t] * s[t]
    sm = softmax over h of w (per column t)
    out[b, h] = sum_t sm[h, t]

v6 implementation (per core, data parallel over batch: 8 cores x 8 batch):
    - E is shipped from host as a bf16 split pair (E = Ehi + Elo exact to
      ~8e-6 rel): same HBM bytes as f32, but the "transposes" become regular
      bf16 matmuls (fast weight load, HAM-warm PE @2.4GHz) instead of
      transpose-mode ops (which run cold and require a permutation rhs).
    - Per t-chunk g=(b,j), per h-chunk i: FOUR matmuls sharing one weight
      load: Et_hi (rhs=I, N=128) + score_hi (rhs=[d_hi|d_lo], N=2, ~30ns)
      then the same pair for lo. Et = (Ehi+Elo)^T accumulates in a
      CONTIGUOUS 2-bank f32 PSUM tile [128,1024]; scores accumulate in a
      separate 1-bank tile [128,8,2] (avoids any bank-crossing matmul).
    - DVE: tiny reduce of score partials -> sneg = -s [128,1]; ONE big
      tensor_scalar: wneg = (Et * sneg) min 3e38 (= -s*Et, f32 -> SBUF),
      accum min -> mneg = -max_h(s*Et). PSUM released here.
    - ACT: e = Exp(-wneg + mneg) from contiguous SBUF, bf16 out,
      accum z = sum_h e.
    - DVE reciprocal r = 1/z; ACT casts to bf16; PE matmul lhsT=r:
      out[1,h] += sum_t r_t e[t,h] accumulated over t-chunks in PSUM.
    - Flattened 64-chunk stream, software-pipelined: recip/cast/final MMs
      run at LAG=3 chunks behind the MM/max/exp stream so no in-order
      engine queue stalls on the cross-engine chain of its own chunk.

Container workaround: this walrus build accepts only ONE sync-wait per
instruction; _split_waits() hoists extra waits onto single-wait Drain
carriers after Tile scheduling.
"""

import ml_dtypes
import numpy as np

import concourse.bass as bass
import concourse.mybir as mybir
from concourse.bass_utils import run_bass_kernel_spmd
from concourse.tile import TileContext

H = 1024
B = 64
T = 1024
N_CORES = 8
B_LOC = B // N_CORES  # 8 batch elements per core
NHC = H // 128        # 8 h-chunks
NTC = T // 128        # 8 t-chunks

F32 = mybir.dt.float32
BF16 = mybir.dt.bfloat16


def _split_waits(nc, max_waits=1):
    """Workaround for this container's walrus: control/compute instructions
    accept only one sync-wait command. Hoist extra waits onto single-wait
    Drain carriers inserted just before the instruction (same engine)."""
    n_new = 0
    for f in nc.m.functions:
        for blk in f.blocks:
            new_insts = []
            for inst in blk.instructions:
                si = inst.sync_info
                if si is not None and si.on_wait is not None and len(si.on_wait) > max_waits:
                    waits = list(si.on_wait)
                    while len(waits) > max_waits:
                        w = waits.pop(0)
                        d = mybir.InstDrain(
                            name=f"I-ws-{nc.next_id()}", ins=[], outs=[]
                        )
                        d.engine = inst.engine
                        d.sync_info = mybir.SyncInfo(on_wait=[w], on_update=[])
                        new_insts.append(d)
                        n_new += 1
                    si.on_wait = waits
                new_insts.append(inst)
            blk.instructions = new_insts
    return n_new


def build_program(split_waits=True):
    """Build the single-core Bass/Tile program (same program runs SPMD on 8 cores)."""
    nc = bass.Bass("TRN2", debug=False, num_devices=N_CORES)
    enc_hi_h = nc.dram_tensor("enc_hi", [H, B_LOC, T], BF16, kind="ExternalInput")
    enc_lo_h = nc.dram_tensor("enc_lo", [H, B_LOC, T], BF16, kind="ExternalInput")
    ident_h = nc.dram_tensor("ident", [128, 128], BF16, kind="ExternalInput")
    decT_h = nc.dram_tensor("decT", [128, B_LOC, NHC, 2], BF16, kind="ExternalInput")
    out_h = nc.dram_tensor("out", [B_LOC, H], F32, kind="ExternalOutput")

    enc_hi = enc_hi_h.ap()
    enc_lo = enc_lo_h.ap()
    ident = ident_h.ap()
    decT = decT_h.ap()
    out = out_h.ap()

    AF = mybir.ActivationFunctionType
    OP = mybir.AluOpType

    with TileContext(nc) as tc:
        with (
            tc.tile_pool(name="constp", bufs=1) as constp,
            tc.tile_pool(name="natp", bufs=3) as natp,
            tc.tile_pool(name="dcp", bufs=2) as dcp,
            tc.tile_pool(name="ep", bufs=5) as ep,
            tc.tile_pool(name="wp", bufs=2) as wp,
            tc.tile_pool(name="smallp", bufs=8) as smallp,
            tc.tile_pool(name="rowp", bufs=2) as rowp,
            tc.tile_pool(name="ps_p", bufs=2, space="PSUM") as ps_p,
            tc.tile_pool(name="ps_s", bufs=2, space="PSUM") as ps_s,
            tc.tile_pool(name="ps_o", bufs=1, space="PSUM") as ps_o,
        ):
            identsb = constp.tile([128, 128], BF16, name="identsb")
            nc.sync.dma_start(out=identsb[:, :], in_=ident)

            # ---- flattened 64-chunk stream, software-pipelined at LAG:
            #      stage A (chunk g):   32 MMs -> s-reduce -> max -> exp
            #      stage B (chunk g-LAG): recip -> bf16 cast -> 2 final MMs
            LAG = 3
            NG = B_LOC * NTC
            state = {}
            o_tiles = {}

            def stage_a(g):
                b, j = divmod(g, NTC)
                if j == 0:
                    nats = []
                    for nm, src in (("nhi", enc_hi), ("nlo", enc_lo)):
                        src_b = src[:, b, :].rearrange("(ii p) t -> p ii t", p=128)
                        nat = natp.tile([128, NHC, T], BF16, name=nm, tag=nm)
                        nc.sync.dma_start(out=nat[:, :, :], in_=src_b[:, :, :])
                        nats.append(nat)
                    dcols = dcp.tile([128, NHC, 2], BF16, name="dcols", tag="dcols")
                    nc.sync.dma_start(out=dcols[:, :, :], in_=decT[:, b, :, :])
                    o_ps = ps_o.tile([1, H], F32, name="o_ps", tag="ps_o")
                    state[("b", b)] = (nats, dcols)
                    o_tiles[b] = o_ps
                nats, dcols = state[("b", b)]

                # Et (contiguous, 2 banks) + score partials (separate bank)
                p_et = ps_p.tile([128, H], F32, name="p_et", tag="ps_p")
                p_s = ps_s.tile([128, NHC, 2], F32, name="p_s", tag="ps_s")
                for i in range(NHC):
                    for part, nat in enumerate(nats):
                        lhsT = nat[:, i, 128 * j : 128 * (j + 1)]
                        nc.tensor.matmul(
                            p_et[:, 128 * i : 128 * (i + 1)],
                            lhsT=lhsT,
                            rhs=identsb[:, :],
                            start=(i % 4 == 0 and part == 0),
                            stop=(i % 4 == 3 and part == 1),
                        )
                        nc.tensor.matmul(
                            p_s[:, i, :],
                            lhsT=lhsT,
                            rhs=dcols[:, i, :],
                            start=(i == 0 and part == 0),
                            stop=(i == NHC - 1 and part == 1),
                        )

                # scores (negated): sneg[t] = -sum_i (s_hi_i + s_lo_i)
                stmp = smallp.tile([128, 2 * NHC], F32, name="stmp", tag="stmp")
                sneg = smallp.tile([128, 1], F32, name="sneg", tag="sneg")
                nc.vector.tensor_scalar(
                    stmp[:, :], p_s[:, :, :], -1.0, 0.0, OP.mult, OP.add,
                    accum_out=sneg[:, :],
                )
                # wneg = -s*Et (f32 SBUF, feeds exp); mneg = -max_h(s*Et)
                wneg = wp.tile([128, H], F32, name="wneg", tag="wneg")
                mneg = smallp.tile([128, 1], F32, name="mneg", tag="mneg")
                nc.vector.tensor_scalar(
                    wneg[:, :], p_et[:, :], sneg[:, :], 3.0e38, OP.mult, OP.min,
                    accum_out=mneg[:, :],
                )
                # e = exp(-wneg + mneg) = exp(s*Et - max); z = sum_h e
                e = ep.tile([128, H], BF16, name="e", tag="e")
                z = smallp.tile([128, 1], F32, name="z", tag="z")
                nc.scalar.activation(
                    e[:, :], wneg[:, :], AF.Exp,
                    bias=mneg[:, :], scale=-1.0, accum_out=z[:, :],
                )
                state[g] = (e, z)

            def stage_b(g):
                b, j = divmod(g, NTC)
                e, z = state.pop(g)
                r = smallp.tile([128, 1], F32, name="r", tag="r")
                nc.vector.reciprocal(r[:, :], z[:, :])
                rl = smallp.tile([128, 1], BF16, name="rl", tag="rl")
                nc.scalar.copy(rl[:, :], r[:, :])
                o_ps = o_tiles[b]
                # out[0, h] += sum_t r_t * e[t, h]
                for half in range(2):
                    nc.tensor.matmul(
                        o_ps[0:1, 512 * half : 512 * half + 512],
                        lhsT=rl[:, :],
                        rhs=e[:, 512 * half : 512 * half + 512],
                        start=(j == 0),
                        stop=(j == NTC - 1),
                    )
                if j == NTC - 1:
                    orow = rowp.tile([1, H], F32, name="orow", tag="orow")
                    nc.scalar.copy(orow[:, :], o_ps[0:1, :])
                    nc.sync.dma_start(out=out[b : b + 1, :], in_=orow[:, :])

            for g in range(NG + LAG):
                if g < NG:
                    stage_a(g)
                if g >= LAG:
                    stage_b(g - LAG)

    if split_waits:
        _split_waits(nc)
    return nc


def make_in_maps(decoder_hidden, encoder_outputs):
    dec = np.ascontiguousarray(np.asarray(decoder_hidden, dtype=np.float32))
    enc = np.ascontiguousarray(np.asarray(encoder_outputs, dtype=np.float32))
    assert dec.shape == (B, H) and enc.shape == (H, B, T)
    bf = ml_dtypes.bfloat16
    enc_hi = enc.astype(bf)
    enc_lo = (enc - enc_hi.astype(np.float32)).astype(bf)
    dec_hi = dec.astype(bf)
    dec_lo = (dec - dec_hi.astype(np.float32)).astype(bf)
    ident = np.eye(128, dtype=bf)

    in_maps = []
    for c in range(N_CORES):
        bsl = slice(c * B_LOC, (c + 1) * B_LOC)
        # decT[p, b, i, 0] = d_hi[b, 128*i + p]; [..., 1] = d_lo
        decT = np.stack(
            [
                dec_hi[bsl].reshape(B_LOC, NHC, 128).transpose(2, 0, 1),
                dec_lo[bsl].reshape(B_LOC, NHC, 128).transpose(2, 0, 1),
            ],
            axis=-1,
        )
        in_maps.append(
            {
                "enc_hi": np.ascontiguousarray(enc_hi[:, bsl, :]),
                "enc_lo": np.ascontiguousarray(enc_lo[:, bsl, :]),
                "ident": ident,
                "decT": np.ascontiguousarray(decT),
            }
        )
    return in_maps


_PROGRAM = None


def kernel(**inputs) -> np.ndarray:
    global _PROGRAM
    if _PROGRAM is None:
        _PROGRAM = build_program()
    in_maps = make_in_maps(inputs["decoder_hidden"], inputs["encoder_outputs"])
    res = run_bass_kernel_spmd(_PROGRAM, in_maps, core_ids=list(range(N_CORES)))
    return np.concatenate([r["out"] for r in res.results], axis=0)


# revision 16
# speedup vs baseline: 1.0831x; 1.0831x over previous
"""Trainium2 Bass kernel for nn_Attn: batched column-softmax attention energies.

Math (per batch element b):
    E = encoder_outputs[:, b, :]            # [H, T]
    d = decoder_hidden[b]                   # [H]
    s = E^T d                               # [T]  (scores)
    w[h, t] = E[h, t] * s[t]
    sm = softmax over h of w (per column t)
    out[b, h] = sum_t sm[h, t]

v3 implementation (per core, data parallel over batch: 8 cores x 8 batch):
    - E is shipped from host as a bf16 split pair (E = Ehi + Elo exactly to
      ~8e-6 rel): same HBM bytes as f32, but the "transposes" become regular
      bf16 matmuls (fast weight load, HAM-warm PE @2.4GHz) instead of
      transpose-mode ops (which run cold and require a permutation rhs).
    - Per t-chunk j and h-chunk i: TWO matmuls (hi, lo) accumulate
      Et[t_part, h] = (Ehi+Elo)^T in f32 PSUM, with rhs = [I | d_hi | d_lo]
      (130 cols): cols 128/129 accumulate E^T d_hi and E^T d_lo -- the
      scores come out of the matmuls for free.
    - PSUM layout avoids bank-crossing matmuls (hardware rejects them):
      3 banks x 3 slots of 130 cols; slot (2,2) is a hole kept at 0
      (memset once per buf). Readers use a [p,3,3,130] strided view; the
      hole contributes max(s*0)=0 to the max (safe: true max >= 0) and
      128*exp(-c) to z (validated: 1.7e-3 rel err on the real data).
    - DVE: tiny reduce of the 18 s-partials -> sT [128,1]; ONE big
      tensor_scalar pass: junk = (Et * sT) max -3e38 with accum max -> m;
      tiny negate -> mneg. (vs baseline's TWO full fp32 passes.)
    - ACT: e = Exp(sT * Et + mneg) read DIRECTLY from PSUM (per-partition
      scale AP), written bf16 to SBUF, accum z = sum_h e.
    - DVE reciprocal r = 1/z, cast bf16; PE matmul lhsT=r: out[1,h] +=
      sum_t r_t e[t,h] accumulated over t-chunks in PSUM.

Container workaround: this walrus build accepts only ONE sync-wait per
instruction; _split_waits() hoists extra waits onto single-wait Drain
carriers after Tile scheduling.
"""

import ml_dtypes
import numpy as np

import concourse.bass as bass
import concourse.mybir as mybir
from concourse.bass_utils import run_bass_kernel_spmd
from concourse.tile import TileContext

H = 1024
B = 64
T = 1024
N_CORES = 8
B_LOC = B // N_CORES  # 8 batch elements per core
NHC = H // 128        # 8 h-chunks
NTC = T // 128        # 8 t-chunks
CW = 130              # slot width: 128 Et cols + d_hi col + d_lo col

F32 = mybir.dt.float32
BF16 = mybir.dt.bfloat16


def _split_waits(nc, max_waits=1):
    """Workaround for this container's walrus: control/compute instructions
    accept only one sync-wait command. Hoist extra waits onto single-wait
    Drain carriers inserted just before the instruction (same engine)."""
    n_new = 0
    for f in nc.m.functions:
        for blk in f.blocks:
            new_insts = []
            for inst in blk.instructions:
                si = inst.sync_info
                if si is not None and si.on_wait is not None and len(si.on_wait) > max_waits:
                    waits = list(si.on_wait)
                    while len(waits) > max_waits:
                        w = waits.pop(0)
                        d = mybir.InstDrain(
                            name=f"I-ws-{nc.next_id()}", ins=[], outs=[]
                        )
                        d.engine = inst.engine
                        d.sync_info = mybir.SyncInfo(on_wait=[w], on_update=[])
                        new_insts.append(d)
                        n_new += 1
                    si.on_wait = waits
                new_insts.append(inst)
            blk.instructions = new_insts
    return n_new


def build_program(split_waits=True):
    """Build the single-core Bass/Tile program (same program runs SPMD on 8 cores)."""
    nc = bass.Bass("TRN2", debug=False, num_devices=N_CORES)
    enc_hi_h = nc.dram_tensor("enc_hi", [H, B_LOC, T], BF16, kind="ExternalInput")
    enc_lo_h = nc.dram_tensor("enc_lo", [H, B_LOC, T], BF16, kind="ExternalInput")
    identblk_h = nc.dram_tensor("identblk", [128, NHC, CW], BF16, kind="ExternalInput")
    decT_h = nc.dram_tensor("decT", [128, B_LOC, NHC, 2], BF16, kind="ExternalInput")
    out_h = nc.dram_tensor("out", [B_LOC, H], F32, kind="ExternalOutput")

    enc_hi = enc_hi_h.ap()
    enc_lo = enc_lo_h.ap()
    identblk = identblk_h.ap()
    decT = decT_h.ap()
    out = out_h.ap()

    AF = mybir.ActivationFunctionType
    OP = mybir.AluOpType

    with TileContext(nc) as tc:
        with (
            tc.tile_pool(name="natp", bufs=3) as natp,
            tc.tile_pool(name="rhsp", bufs=2) as rhsp,
            tc.tile_pool(name="ep", bufs=5) as ep,
            tc.tile_pool(name="junkp", bufs=2) as junkp,
            tc.tile_pool(name="smallp", bufs=8) as smallp,
            tc.tile_pool(name="rowp", bufs=2) as rowp,
            tc.tile_pool(name="ps_p", bufs=2, space="PSUM") as ps_p,
            tc.tile_pool(name="ps_o", bufs=1, space="PSUM") as ps_o,
        ):
            # identity blocks with zeroed d-columns, loaded once into both
            # rhs bufs; per-b DMA then fills only the 16 d-columns.
            rhsbufs = []
            for v in range(2):
                rhsb = rhsp.tile([128, NHC, CW], BF16, name=f"rhsb{v}", tag="rhsb")
                nc.sync.dma_start(out=rhsb[:, :, :], in_=identblk[:, :, :])
                rhsbufs.append(rhsb)

            # fixed PSUM tiles (not pool-cycled) so the hole slot (2,2),
            # zeroed once here, legally persists across iterations
            psbufs = []
            for v in range(2):
                p_ps = ps_p.tile([128, 3, 512], F32, name=f"p_ps{v}", tag="ps_p")
                nc.vector.memset(p_ps[:, 2, 2 * CW : 3 * CW], 0.0)
                psbufs.append(p_ps)

            # ---- flattened 64-chunk stream, software-pipelined at lag 2:
            #      stage A (chunk g):   16 MMs -> s-reduce -> max -> neg -> exp
            #      stage B (chunk g-2): recip -> bf16 cast -> 2 final MMs
            # Lag keeps each in-order engine queue from stalling on the
            # cross-engine chain of its own chunk.
            LAG = 3
            NG = B_LOC * NTC
            state = {}   # g -> (e, rl_pending...) for stage B
            o_tiles = {}

            def stage_a(g):
                b, j = divmod(g, NTC)
                if j == 0:
                    # ---- per-b loads (prefetched ~8 chunks ahead by Tile).
                    # b=0 is split at t=128 so the first chunk's matmuls can
                    # start after ~1/8 of the load instead of the full 4MB.
                    nats = []
                    for nm, src in (("nhi", enc_hi), ("nlo", enc_lo)):
                        src_b = src[:, b, :].rearrange("(ii p) t -> p ii t", p=128)
                        nat = natp.tile([128, NHC, T], BF16, name=nm, tag=nm)
                        if b == 0:
                            nc.sync.dma_start(
                                out=nat[:, :, 0:128], in_=src_b[:, :, 0:128]
                            )
                            nc.sync.dma_start(
                                out=nat[:, :, 128:T], in_=src_b[:, :, 128:T]
                            )
                        else:
                            nc.sync.dma_start(out=nat[:, :, :], in_=src_b[:, :, :])
                        nats.append(nat)
                    rhsb = rhsbufs[b % 2]
                    nc.sync.dma_start(out=rhsb[:, :, 128:130], in_=decT[:, b, :, :])
                    o_ps = ps_o.tile([1, H], F32, name="o_ps", tag="ps_o")
                    state[("b", b)] = (nats, rhsb)
                    o_tiles[b] = o_ps
                nats, rhsb = state[("b", b)]

                p_ps = psbufs[g % 2]
                for i in range(NHC):
                    k, m = divmod(i, 3)
                    for part, nat in enumerate(nats):
                        nc.tensor.matmul(
                            p_ps[:, k, CW * m : CW * m + CW],
                            lhsT=nat[:, i, 128 * j : 128 * (j + 1)],
                            rhs=rhsb[:, i, :],
                            start=(m == 0 and part == 0),
                            stop=((m == 2 or i == NHC - 1) and part == 1),
                        )
                # strided [p, bank, slot, col] view of the 9 slots
                v9 = p_ps[:, :, 0 : 3 * CW].rearrange("p k (m c) -> p k m c", c=CW)
                et = v9[:, :, :, 0:128]        # [128, 3, 3, 128]
                scol = v9[:, :, :, 128:130]    # [128, 3, 3, 2]

                # scores (negated): sneg[t] = -sum_i (s_hi_i + s_lo_i)
                stmp = smallp.tile([128, 18], F32, name="stmp", tag="stmp")
                sneg = smallp.tile([128, 1], F32, name="sneg", tag="sneg")
                nc.vector.tensor_scalar(
                    stmp[:, :], scol, -1.0, 0.0, OP.mult, OP.add, accum_out=sneg[:, :]
                )
                # wneg = -s*Et PACKED [128, 1024] f32 SBUF (hole excluded), in
                # two ops (banks 0-1 -> cols 0:768, bank 2 slots 0-1 ->
                # 768:1024); per-op min accums combined -> mneg = -max_h(s*Et)
                wneg = junkp.tile([128, H], F32, name="wneg", tag="wneg")
                mnA = smallp.tile([128, 1], F32, name="mnA", tag="mnA")
                mnB = smallp.tile([128, 1], F32, name="mnB", tag="mnB")
                nc.vector.tensor_scalar(
                    wneg[:, 0:768], et[:, 0:2, :, :], sneg[:, :], 3.0e38,
                    OP.mult, OP.min, accum_out=mnA[:, :],
                )
                nc.vector.tensor_scalar(
                    wneg[:, 768:1024], et[:, 2, 0:2, :], sneg[:, :], 3.0e38,
                    OP.mult, OP.min, accum_out=mnB[:, :],
                )
                mneg = smallp.tile([128, 1], F32, name="mneg", tag="mneg")
                nc.vector.tensor_scalar(
                    mneg[:, :], mnA[:, :], mnB[:, :], None, OP.min, OP.bypass
                )
                # e = exp(-wneg + mneg) = exp(s*Et - max); z = sum_h e (exact:
                # wneg is packed, no hole contribution). Contiguous SBUF read;
                # the PSUM tile is released after the wneg passes.
                e = ep.tile([128, H], BF16, name="e", tag="e")
                z = smallp.tile([128, 1], F32, name="z", tag="z")
                nc.scalar.activation(
                    e[:, :], wneg[:, :], AF.Exp,
                    bias=mneg[:, :], scale=-1.0, accum_out=z[:, :],
                )
                state[g] = (e, z)

            def stage_b(g):
                b, j = divmod(g, NTC)
                e, z = state.pop(g)
                rl = smallp.tile([128, 1], BF16, name="rl", tag="rl")
                with nc.allow_low_precision("r feeds a bf16 matmul lhsT"):
                    nc.vector.reciprocal(rl[:, :], z[:, :])
                o_ps = o_tiles[b]
                # out[0, h] += sum_t r_t * e[t, h]
                for half in range(2):
                    nc.tensor.matmul(
                        o_ps[0:1, 512 * half : 512 * half + 512],
                        lhsT=rl[:, :],
                        rhs=e[:, 512 * half : 512 * half + 512],
                        start=(j == 0),
                        stop=(j == NTC - 1),
                    )
                if j == NTC - 1:
                    orow = rowp.tile([1, H], F32, name="orow", tag="orow")
                    nc.scalar.copy(orow[:, :], o_ps[0:1, :])
                    nc.sync.dma_start(out=out[b : b + 1, :], in_=orow[:, :])

            for g in range(NG + LAG):
                if g < NG:
                    stage_a(g)
                if g >= LAG:
                    stage_b(g - LAG)

    if split_waits:
        _split_waits(nc)
    return nc


def make_in_maps(decoder_hidden, encoder_outputs):
    dec = np.ascontiguousarray(np.asarray(decoder_hidden, dtype=np.float32))
    enc = np.ascontiguousarray(np.asarray(encoder_outputs, dtype=np.float32))
    assert dec.shape == (B, H) and enc.shape == (H, B, T)
    bf = ml_dtypes.bfloat16
    enc_hi = enc.astype(bf)
    enc_lo = (enc - enc_hi.astype(np.float32)).astype(bf)
    dec_hi = dec.astype(bf)
    dec_lo = (dec - dec_hi.astype(np.float32)).astype(bf)

    identblk = np.zeros((128, NHC, CW), dtype=bf)
    eye = np.eye(128, dtype=bf)
    for i in range(NHC):
        identblk[:, i, 0:128] = eye

    in_maps = []
    for c in range(N_CORES):
        bsl = slice(c * B_LOC, (c + 1) * B_LOC)
        # decT[p, b, i, 0] = d_hi[b, 128*i + p]; [..., 1] = d_lo
        decT = np.stack(
            [
                dec_hi[bsl].reshape(B_LOC, NHC, 128).transpose(2, 0, 1),
                dec_lo[bsl].reshape(B_LOC, NHC, 128).transpose(2, 0, 1),
            ],
            axis=-1,
        )
        in_maps.append(
            {
                "enc_hi": np.ascontiguousarray(enc_hi[:, bsl, :]),
                "enc_lo": np.ascontiguousarray(enc_lo[:, bsl, :]),
                "identblk": identblk,
                "decT": np.ascontiguousarray(decT),
            }
        )
    return in_maps


_PROGRAM = None


def kernel(**inputs) -> np.ndarray:
    global _PROGRAM
    if _PROGRAM is None:
        _PROGRAM = build_program()
    in_maps = make_in_maps(inputs["decoder_hidden"], inputs["encoder_outputs"])
    res = run_bass_kernel_spmd(_PROGRAM, in_maps, core_ids=list(range(N_CORES)))
    return np.concatenate([r["out"] for r in res.results], axis=0)
